# revision 1
# baseline (speedup 1.0000x reference)
"""Causal multi-head self-attention with RoPE on 8 TRN2 NeuronCores.

Sharding: data-parallel over batch (4) x tensor-parallel over heads (16 -> 2
groups of 8).  Core c handles batch c//2, head group c%2.  Each core computes
its 8 heads' attention and a partial O-projection (512 of the 1024 contraction
dims); the host sums the two partials per batch element.
"""

import os
import sys

import numpy as np

if "/opt/trn_rl_repo" not in sys.path:
    sys.path.insert(0, "/opt/trn_rl_repo")

D_MODEL = 1024
NUM_HEADS = 16
THETA = 10000.0
B, S = 4, 2048
DK = 64
HALF = DK // 2
P = 128
N_CORES = 8
HPC = 8                 # heads per core
DOUT = HPC * DK         # 512 per-core projected dims
KT = D_MODEL // P       # 8 contraction tiles
NSEQ = S // P           # 16 seq tiles of 128
NQB = S // 512          # 4 query blocks of 512
SCALE = 1.0 / np.sqrt(DK)

_CACHE = {}


def _build():
    """Build + compile the per-core Bass module (same program on all cores)."""
    import concourse.bass as bass
    import concourse.bacc as bacc
    import concourse.tile as tile
    import concourse.mybir as mybir
    from contextlib import ExitStack

    f32 = mybir.dt.float32
    bf16 = mybir.dt.bfloat16
    Exp = mybir.ActivationFunctionType.Exp

    nc = bacc.Bacc("TRN2", target_bir_lowering=False, debug=False,
                   enable_asserts=False, num_devices=N_CORES)

    xT = nc.dram_tensor("xT", [D_MODEL, S], bf16, kind="ExternalInput")
    wq = nc.dram_tensor("wq", [D_MODEL, DOUT], bf16, kind="ExternalInput")
    wk = nc.dram_tensor("wk", [D_MODEL, DOUT], bf16, kind="ExternalInput")
    wv = nc.dram_tensor("wv", [D_MODEL, DOUT], bf16, kind="ExternalInput")
    wo = nc.dram_tensor("wo", [DOUT, D_MODEL], bf16, kind="ExternalInput")
    cosn = nc.dram_tensor("cosn", [S, DK], f32, kind="ExternalInput")
    sinn = nc.dram_tensor("sinn", [S, DK], f32, kind="ExternalInput")
    maskt = nc.dram_tensor("maskt", [P, 4 * 512], bf16, kind="ExternalInput")
    ident = nc.dram_tensor("ident", [P, P], bf16, kind="ExternalInput")
    out = nc.dram_tensor("out", [S, D_MODEL], f32, kind="ExternalOutput")

    def rep8(ap):
        # replicate a [128, 64] tile 8x along free dim -> logical [128, 512]
        return bass.AP(tensor=ap.tensor, offset=ap.offset,
                       ap=[ap.ap[0], [0, HPC], [1, DK]])

    def pairswap(ap):
        # free-dim pair swap of a [128, 512] tile: (0,1,2,3,..)->(1,0,3,2,..)
        return bass.AP(tensor=ap.tensor, offset=ap.offset + 1,
                       ap=[ap.ap[0], [2, 256], [-1, 2]])

    with tile.TileContext(nc) as tc, ExitStack() as top:
        persist = top.enter_context(tc.tile_pool(name="persist", bufs=1))
        # psum pools (8 banks total): proj/oproj share 2, transpose 2,
        # scores 2, attention-accumulate 2
        mm_ps = top.enter_context(tc.tile_pool(name="mm_ps", bufs=2, space="PSUM"))
        tr_ps = top.enter_context(tc.tile_pool(name="tr_ps", bufs=2, space="PSUM"))
        sc_ps = top.enter_context(tc.tile_pool(name="sc_ps", bufs=2, space="PSUM"))
        av_ps = top.enter_context(tc.tile_pool(name="av_ps", bufs=2, space="PSUM"))
        ropet = top.enter_context(tc.tile_pool(name="ropet", bufs=2))
        natp = top.enter_context(tc.tile_pool(name="natp", bufs=4))
        ptp = top.enter_context(tc.tile_pool(name="ptp", bufs=4))
        rcpp = top.enter_context(tc.tile_pool(name="rcpp", bufs=2))
        rmatp = top.enter_context(tc.tile_pool(name="rmatp", bufs=2))
        ostg = top.enter_context(tc.tile_pool(name="ostg", bufs=3))

        # ---- persistent SBUF arrays ----
        # DMA emission order matters (single queue): interleave x/wq tiles so
        # the first Q-proj matmuls can start after ~1MB instead of ~5MB, then
        # rope tables, then the rest in first-use order.
        x_sb = [persist.tile([P, S], bf16, tag=f"x{k}", name=f"x{k}")
                for k in range(KT)]
        w_sb = {nm: [persist.tile([P, DOUT], bf16, tag=f"{nm}{k}",
                                  name=f"{nm}{k}") for k in range(KT)]
                for nm in ("wq", "wk", "wv")}
        wo_sb = [persist.tile([P, D_MODEL], bf16, tag=f"wo{k}", name=f"wo{k}")
                 for k in range(DOUT // P)]
        cos_sb = [persist.tile([P, DK], f32, tag=f"cos{m}", name=f"cos{m}")
                  for m in range(NSEQ)]
        sin_sb = [persist.tile([P, DK], f32, tag=f"sin{m}", name=f"sin{m}")
                  for m in range(NSEQ)]
        mask_sb = persist.tile([P, 4 * 512], bf16, tag="mask", name="mask")
        id_sb = persist.tile([P, P], bf16, tag="ident", name="ident")

        for k in range(KT):
            nc.sync.dma_start(out=x_sb[k], in_=xT[k * P:(k + 1) * P, :])
            nc.sync.dma_start(out=w_sb["wq"][k], in_=wq[k * P:(k + 1) * P, :])
        for m in range(8):
            nc.sync.dma_start(out=cos_sb[m], in_=cosn[m * P:(m + 1) * P, :])
            nc.sync.dma_start(out=sin_sb[m], in_=sinn[m * P:(m + 1) * P, :])
        nc.sync.dma_start(out=id_sb, in_=ident[:, :])
        for k in range(KT):
            nc.sync.dma_start(out=w_sb["wk"][k], in_=wk[k * P:(k + 1) * P, :])
        for m in range(8, NSEQ):
            nc.sync.dma_start(out=cos_sb[m], in_=cosn[m * P:(m + 1) * P, :])
            nc.sync.dma_start(out=sin_sb[m], in_=sinn[m * P:(m + 1) * P, :])
        for k in range(KT):
            nc.sync.dma_start(out=w_sb["wv"][k], in_=wv[k * P:(k + 1) * P, :])
        nc.sync.dma_start(out=mask_sb, in_=maskt[:, :])
        for k in range(DOUT // P):
            nc.sync.dma_start(out=wo_sb[k], in_=wo[k * P:(k + 1) * P, :])

        # outputs of phase A
        qt_sb = [persist.tile([P, S], bf16, tag=f"qt{d}", name=f"qt{d}")
                 for d in range(4)]
        kt_sb = [persist.tile([P, S], bf16, tag=f"kt{d}", name=f"kt{d}")
                 for d in range(4)]
        v_sb = [persist.tile([P, HPC * (DK + 1)], bf16, tag=f"v{t}", name=f"v{t}")
                for t in range(NSEQ)]
        ot_sb = [persist.tile([P, S], bf16, tag=f"ot{d}", name=f"ot{d}")
                 for d in range(4)]

        # ---- group-interleaved pipeline over seq groups g (4 m-tiles each) --

        def proj_group(g):
            """Q/K/V projections + rope + transpose for m in [4g, 4g+4)."""
            for nm, dst in (("wq", qt_sb), ("wk", kt_sb)):
                pend = None          # lag-1 transpose drain: (trt, m)
                for m in range(4 * g, 4 * g + 4):
                    ps = mm_ps.tile([P, DOUT], f32, tag="mm", name="mm")
                    for k in range(KT):
                        nc.tensor.matmul(ps, x_sb[k][:, m * P:(m + 1) * P],
                                         w_sb[nm][k], start=(k == 0),
                                         stop=(k == KT - 1))
                    t1 = ropet.tile([P, DOUT], f32, tag="rt1", name="rt1")
                    t2 = ropet.tile([P, DOUT], f32, tag="rt2", name="rt2")
                    nc.vector.tensor_mul(t1, ps, rep8(cos_sb[m]))
                    nc.vector.tensor_mul(t2, pairswap(ps), rep8(sin_sb[m]))
                    nat = natp.tile([P, DOUT], bf16, tag="nat", name="nat")
                    nc.vector.tensor_add(nat, t1, t2)
                    if pend is not None:
                        ptr, pm = pend
                        for d in range(4):
                            nc.vector.tensor_copy(
                                dst[d][:, pm * P:(pm + 1) * P],
                                ptr[:, d * P:(d + 1) * P])
                    trt = tr_ps.tile([P, 512], bf16, tag="tr", name="trt")
                    for d in range(4):
                        nc.tensor.transpose(trt[:, d * P:(d + 1) * P],
                                            nat[:, d * P:(d + 1) * P], id_sb)
                    pend = (trt, m)
                ptr, pm = pend
                for d in range(4):
                    nc.vector.tensor_copy(dst[d][:, pm * P:(pm + 1) * P],
                                          ptr[:, d * P:(d + 1) * P])
            for m in range(4 * g, 4 * g + 4):
                ps = mm_ps.tile([P, DOUT], f32, tag="mm", name="mm")
                for k in range(KT):
                    nc.tensor.matmul(ps, x_sb[k][:, m * P:(m + 1) * P],
                                     w_sb["wv"][k], start=(k == 0),
                                     stop=(k == KT - 1))
                vt = v_sb[m]
                ones_ap = bass.AP(tensor=vt.tensor, offset=vt.offset + DK,
                                  ap=[vt.ap[0], [DK + 1, HPC]])
                nc.gpsimd.memset(ones_ap, 1.0)
                vcols = bass.AP(tensor=vt.tensor, offset=vt.offset,
                                ap=[vt.ap[0], [DK + 1, HPC], [1, DK]])
                nc.scalar.copy(vcols, ps)

        def attn_group(g):
            """Attention for query block qb=g over all heads.

            Diagonal kv-tiles (t in [4g, 4g+4)) only have valid scores for
            q-cols >= 128*(t%4): scores+exp are trimmed to that span, and the
            full-width 0/1 mask-mul zeroes both the stale prefix and the
            intra-tile upper triangle of pt before the full-width AV matmul.
            """
            cols = slice(g * 512, (g + 1) * 512)
            for h in range(HPC):
                db, po = h // 2, (h % 2) * DK
                av = av_ps.tile([DK + 1, 512], f32, tag="av", name="av")
                nt = 4 * g + 4
                for t in range(nt):
                    v = t - 4 * g
                    c0 = 128 * v if v >= 0 else 0
                    sc = sc_ps.tile([P, 512], f32, tag="sc", name="sc")
                    nc.tensor.matmul(
                        sc[:, c0:], kt_sb[db][po:po + DK, t * P:(t + 1) * P],
                        qt_sb[db][po:po + DK, g * 512 + c0:(g + 1) * 512],
                        start=True, stop=True)
                    pt = ptp.tile([P, 512], bf16, tag="pt", name="pt")
                    nc.scalar.activation(pt[:, c0:], sc[:, c0:], Exp)
                    if v >= 0:
                        nc.vector.tensor_mul(
                            pt, pt, mask_sb[:, v * 512:(v + 1) * 512])
                    nc.tensor.matmul(
                        av, v_sb[t][:, h * (DK + 1):(h + 1) * (DK + 1)],
                        pt, start=(t == 0), stop=(t == nt - 1))
                rcp = rcpp.tile([1, 512], f32, tag="rcp", name="rcp")
                nc.vector.reciprocal(rcp, av[DK:DK + 1, :])
                rmat = rmatp.tile([DK, 512], f32, tag="rmat", name="rmat")
                nc.gpsimd.partition_broadcast(rmat, rcp, channels=DK)
                nc.vector.tensor_mul(ot_sb[db][po:po + DK, cols],
                                     av[0:DK, :], rmat)

        def oproj_group(g):
            for m in range(4 * g, 4 * g + 4):
                for nb in range(2):
                    ps = mm_ps.tile([P, 512], f32, tag="mm", name="mm")
                    for k in range(4):
                        nc.tensor.matmul(
                            ps, ot_sb[k][:, m * P:(m + 1) * P],
                            wo_sb[k][:, nb * 512:(nb + 1) * 512],
                            start=(k == 0), stop=(k == 3))
                    og = ostg.tile([P, 512], f32, tag="og", name="og")
                    nc.vector.tensor_copy(og, ps)
                    nc.sync.dma_start(
                        out=out[m * P:(m + 1) * P, nb * 512:(nb + 1) * 512],
                        in_=og)

        # zero the pt pool slots once: trimmed exp leaves stale prefixes that
        # the mask-mul reads (0 * garbage must not be 0 * NaN)
        for _ in range(4):
            ptz = ptp.tile([P, 512], bf16, tag="pt", name="ptz")
            nc.gpsimd.memset(ptz, 0.0)

        # software-staged emission: keep PE fed with proj work while the
        # ACT-heavy attention of earlier groups drains
        proj_group(0)
        proj_group(1)
        for g in range(4):
            attn_group(g)
            if g + 2 < 4:
                proj_group(g + 2)
            oproj_group(g)

    nc.compile()
    return nc


def _get_nc():
    if "nc" not in _CACHE:
        _CACHE["nc"] = _build()
    return _CACHE["nc"]


def _prep_core_inputs(q_proj_weight, k_proj_weight, v_proj_weight,
                      o_proj_weight, in_features, token_positions):
    """Host-side sharding: returns the list of 8 per-core input dicts."""
    import ml_dtypes
    bf = ml_dtypes.bfloat16

    x = np.asarray(in_features, np.float32)
    wqf = np.asarray(q_proj_weight, np.float32)
    wkf = np.asarray(k_proj_weight, np.float32)
    wvf = np.asarray(v_proj_weight, np.float32)
    wof = np.asarray(o_proj_weight, np.float32)
    tp = np.asarray(token_positions).astype(np.float64)

    inv = 1.0 / (THETA ** (np.arange(HALF, dtype=np.float64) / HALF))
    fr = tp[:, None] * inv[None, :]                       # [S, 32]
    cosn = np.repeat(np.cos(fr), 2, axis=1).astype(np.float32)  # [S, 64]
    sg = np.tile(np.array([-1.0, 1.0]), HALF)[None, :]
    sinn = (np.repeat(np.sin(fr), 2, axis=1) * sg).astype(np.float32)

    kv = np.arange(P)[:, None]
    qc = np.arange(512)[None, :]
    maskt = np.concatenate(
        [(qc >= 128 * v + kv) for v in range(4)], axis=1).astype(bf)

    identity = np.eye(P, dtype=bf)

    in_maps = []
    for c in range(N_CORES):
        b, hg = c // 2, c % 2
        rows = slice(hg * DOUT, (hg + 1) * DOUT)
        wv_s = wvf[rows].T.astype(bf)                      # [1024, 512]
        in_maps.append({
            "xT": np.ascontiguousarray(x[b].T).astype(bf),
            "wq": np.ascontiguousarray((wqf[rows] * SCALE).T).astype(bf),
            "wk": np.ascontiguousarray(wkf[rows].T).astype(bf),
            "wv": np.ascontiguousarray(wv_s),
            "wo": np.ascontiguousarray(wof[:, rows].T).astype(bf),
            "cosn": cosn,
            "sinn": sinn,
            "maskt": maskt,
            "ident": identity,
        })
    return in_maps


def kernel(q_proj_weight, k_proj_weight, v_proj_weight, o_proj_weight,
           in_features, token_positions):
    from concourse.bass_utils import run_bass_kernel_spmd

    nc = _get_nc()
    in_maps = _prep_core_inputs(q_proj_weight, k_proj_weight, v_proj_weight,
                                o_proj_weight, in_features, token_positions)
    trace = bool(int(os.environ.get("KBENCH_TRACE", "0")))
    res = run_bass_kernel_spmd(nc, in_maps, list(range(N_CORES)), trace=trace)
    _CACHE["last_results"] = res
    if res.exec_time_ns is not None:
        _CACHE["exec_time_ns"] = res.exec_time_ns

    outp = np.empty((B, S, D_MODEL), np.float32)
    for b in range(B):
        outp[b] = res.results[2 * b]["out"] + res.results[2 * b + 1]["out"]
    return outp



# revision 2
# speedup vs baseline: 33.2582x; 33.2582x over previous
"""Causal multi-head self-attention with RoPE on 8 TRN2 NeuronCores.

Sharding: data-parallel over batch (4) x tensor-parallel over heads (16 -> 2
groups of 8).  Core c handles batch c//2, head group c%2.  Each core computes
its 8 heads' attention and a partial O-projection (512 of the 1024 contraction
dims); the host sums the two partials per batch element.

All per-core inputs are packed into ONE bf16 dram blob (cuts per-call arg
marshalling); output is bf16 (halves store+fetch), upcast+summed on host.
"""

import os
import sys

import numpy as np

if "/opt/trn_rl_repo" not in sys.path:
    sys.path.insert(0, "/opt/trn_rl_repo")

D_MODEL = 1024
NUM_HEADS = 16
THETA = 10000.0
B, S = 4, 2048
DK = 64
HALF = DK // 2
P = 128
N_CORES = 8
HPC = 8                 # heads per core
DOUT = HPC * DK         # 512 per-core projected dims
KT = D_MODEL // P       # 8 contraction tiles
NSEQ = S // P           # 16 seq tiles of 128
NQB = S // 512          # 4 query blocks of 512
SCALE = 1.0 / np.sqrt(DK)

# packed blob layout (bf16 element offsets)
_SEGS = [
    ("xT", D_MODEL * S),           # [1024, 2048]
    ("wq", D_MODEL * DOUT),        # [1024, 512]
    ("wk", D_MODEL * DOUT),
    ("wv", D_MODEL * DOUT),
    ("cos", S * DK),               # [2048, 64]
    ("sin", S * DK),
    ("mask", P * 4 * 512),         # [128, 2048]
    ("ident", P * P),              # [128, 128]
    ("wo", DOUT * D_MODEL),        # [512, 1024]
]
_OFF = {}
_t = 0
for _nm, _sz in _SEGS:
    _OFF[_nm] = _t
    _t += _sz
BLOB_N = _t

_CACHE = {}


def _build(reps=1):
    """Build + compile the per-core Bass module (same program on all cores).

    reps > 1 repeats the whole body (input DMAs + compute + stores) that many
    times in one NEFF; used by the benchmark to measure marginal exec time.
    """
    import concourse.bass as bass
    import concourse.bacc as bacc
    import concourse.tile as tile
    import concourse.mybir as mybir
    from contextlib import ExitStack

    f32 = mybir.dt.float32
    bf16 = mybir.dt.bfloat16
    Exp = mybir.ActivationFunctionType.Exp

    nc = bacc.Bacc("TRN2", target_bir_lowering=False, debug=False,
                   enable_asserts=False, num_devices=N_CORES)

    blob = nc.dram_tensor("blob", [BLOB_N], bf16, kind="ExternalInput")
    out = nc.dram_tensor("out", [S, D_MODEL], bf16, kind="ExternalOutput")
    blob_t = blob[0:1].tensor

    def brows(name, row0, nrows, rowlen):
        # [nrows, rowlen] 2-D window into the flat blob segment `name`
        return bass.AP(tensor=blob_t, offset=_OFF[name] + row0 * rowlen,
                       ap=[[rowlen, nrows], [1, rowlen]])

    def rep8(ap):
        # replicate a [128, 64] tile 8x along free dim -> logical [128, 512]
        return bass.AP(tensor=ap.tensor, offset=ap.offset,
                       ap=[ap.ap[0], [0, HPC], [1, DK]])

    def pairswap(ap):
        # free-dim pair swap of a [128, 512] tile: (0,1,2,3,..)->(1,0,3,2,..)
        return bass.AP(tensor=ap.tensor, offset=ap.offset + 1,
                       ap=[ap.ap[0], [2, 256], [-1, 2]])

    with tile.TileContext(nc) as tc, ExitStack() as top:
        persist = top.enter_context(tc.tile_pool(name="persist", bufs=1))
        # psum pools (8 banks total): proj/oproj share 2, transpose 2,
        # scores 2, attention-accumulate 2
        mm_ps = top.enter_context(tc.tile_pool(name="mm_ps", bufs=2, space="PSUM"))
        tr_ps = top.enter_context(tc.tile_pool(name="tr_ps", bufs=2, space="PSUM"))
        sc_ps = top.enter_context(tc.tile_pool(name="sc_ps", bufs=2, space="PSUM"))
        av_ps = top.enter_context(tc.tile_pool(name="av_ps", bufs=2, space="PSUM"))
        ropet = top.enter_context(tc.tile_pool(name="ropet", bufs=2))
        natp = top.enter_context(tc.tile_pool(name="natp", bufs=4))
        ptp = top.enter_context(tc.tile_pool(name="ptp", bufs=4))
        rcpp = top.enter_context(tc.tile_pool(name="rcpp", bufs=2))
        rmatp = top.enter_context(tc.tile_pool(name="rmatp", bufs=2))
        ostg = top.enter_context(tc.tile_pool(name="ostg", bufs=3))

        # zero the pt pool slots once: trimmed exp leaves stale prefixes that
        # the mask-mul reads (0 * garbage must not be 0 * NaN)
        for _ in range(4):
            ptz = ptp.tile([P, 512], bf16, tag="pt", name="ptz")
            nc.gpsimd.memset(ptz, 0.0)

        def emit_body():
            # ---- persistent SBUF arrays ----
            # DMA emission order matters (single queue): interleave x/wq
            # tiles so the first Q-proj matmuls can start early, then rope
            # tables, then the rest in first-use order.
            x_sb = [persist.tile([P, S], bf16, tag=f"x{k}", name=f"x{k}")
                    for k in range(KT)]
            w_sb = {nm: [persist.tile([P, DOUT], bf16, tag=f"{nm}{k}",
                                      name=f"{nm}{k}") for k in range(KT)]
                    for nm in ("wq", "wk", "wv")}
            wo_sb = [persist.tile([P, D_MODEL], bf16, tag=f"wo{k}",
                                  name=f"wo{k}") for k in range(DOUT // P)]
            cos_sb = [persist.tile([P, DK], bf16, tag=f"cos{m}",
                                   name=f"cos{m}") for m in range(NSEQ)]
            sin_sb = [persist.tile([P, DK], bf16, tag=f"sin{m}",
                                   name=f"sin{m}") for m in range(NSEQ)]
            mask_sb = persist.tile([P, 4 * 512], bf16, tag="mask", name="mask")
            id_sb = persist.tile([P, P], bf16, tag="ident", name="ident")

            for k in range(KT):
                nc.sync.dma_start(out=x_sb[k], in_=brows("xT", k * P, P, S))
                nc.sync.dma_start(out=w_sb["wq"][k],
                                  in_=brows("wq", k * P, P, DOUT))
            for m in range(8):
                nc.sync.dma_start(out=cos_sb[m], in_=brows("cos", m * P, P, DK))
                nc.sync.dma_start(out=sin_sb[m], in_=brows("sin", m * P, P, DK))
            nc.sync.dma_start(out=id_sb, in_=brows("ident", 0, P, P))
            for k in range(KT):
                nc.sync.dma_start(out=w_sb["wk"][k],
                                  in_=brows("wk", k * P, P, DOUT))
            for m in range(8, NSEQ):
                nc.sync.dma_start(out=cos_sb[m], in_=brows("cos", m * P, P, DK))
                nc.sync.dma_start(out=sin_sb[m], in_=brows("sin", m * P, P, DK))
            for k in range(KT):
                nc.sync.dma_start(out=w_sb["wv"][k],
                                  in_=brows("wv", k * P, P, DOUT))
            nc.sync.dma_start(out=mask_sb, in_=brows("mask", 0, P, 4 * 512))
            for k in range(DOUT // P):
                nc.sync.dma_start(out=wo_sb[k],
                                  in_=brows("wo", k * P, P, D_MODEL))

            # outputs of phase A
            qt_sb = [persist.tile([P, S], bf16, tag=f"qt{d}", name=f"qt{d}")
                     for d in range(4)]
            kt_sb = [persist.tile([P, S], bf16, tag=f"kt{d}", name=f"kt{d}")
                     for d in range(4)]
            v_sb = [persist.tile([P, HPC * (DK + 1)], bf16, tag=f"v{t}",
                                 name=f"v{t}") for t in range(NSEQ)]
            ot_sb = [persist.tile([P, S], bf16, tag=f"ot{d}", name=f"ot{d}")
                     for d in range(4)]

            def proj_group(g):
                """Q/K/V projections + rope + transpose, m in [4g, 4g+4)."""
                for nm, dst in (("wq", qt_sb), ("wk", kt_sb)):
                    pend = None          # lag-1 transpose drain: (trt, m)
                    for m in range(4 * g, 4 * g + 4):
                        ps = mm_ps.tile([P, DOUT], f32, tag="mm", name="mm")
                        for k in range(KT):
                            nc.tensor.matmul(ps, x_sb[k][:, m * P:(m + 1) * P],
                                             w_sb[nm][k], start=(k == 0),
                                             stop=(k == KT - 1))
                        t1 = ropet.tile([P, DOUT], f32, tag="rt1", name="rt1")
                        t2 = ropet.tile([P, DOUT], f32, tag="rt2", name="rt2")
                        nc.vector.tensor_mul(t1, ps, rep8(cos_sb[m]))
                        nc.vector.tensor_mul(t2, pairswap(ps), rep8(sin_sb[m]))
                        nat = natp.tile([P, DOUT], bf16, tag="nat", name="nat")
                        nc.vector.tensor_add(nat, t1, t2)
                        if pend is not None:
                            ptr, pm = pend
                            for d in range(4):
                                nc.vector.tensor_copy(
                                    dst[d][:, pm * P:(pm + 1) * P],
                                    ptr[:, d * P:(d + 1) * P])
                        trt = tr_ps.tile([P, 512], bf16, tag="tr", name="trt")
                        for d in range(4):
                            nc.tensor.transpose(trt[:, d * P:(d + 1) * P],
                                                nat[:, d * P:(d + 1) * P],
                                                id_sb)
                        pend = (trt, m)
                    ptr, pm = pend
                    for d in range(4):
                        nc.vector.tensor_copy(dst[d][:, pm * P:(pm + 1) * P],
                                              ptr[:, d * P:(d + 1) * P])
                for m in range(4 * g, 4 * g + 4):
                    ps = mm_ps.tile([P, DOUT], f32, tag="mm", name="mm")
                    for k in range(KT):
                        nc.tensor.matmul(ps, x_sb[k][:, m * P:(m + 1) * P],
                                         w_sb["wv"][k], start=(k == 0),
                                         stop=(k == KT - 1))
                    vt = v_sb[m]
                    ones_ap = bass.AP(tensor=vt.tensor, offset=vt.offset + DK,
                                      ap=[vt.ap[0], [DK + 1, HPC]])
                    nc.gpsimd.memset(ones_ap, 1.0)
                    vcols = bass.AP(tensor=vt.tensor, offset=vt.offset,
                                    ap=[vt.ap[0], [DK + 1, HPC], [1, DK]])
                    nc.scalar.copy(vcols, ps)

            def attn_group(g):
                """Attention for query block qb=g over all heads.

                Diagonal kv-tiles (t in [4g, 4g+4)) only have valid scores
                for q-cols >= 128*(t%4): scores+exp are trimmed to that span,
                and the full-width 0/1 mask-mul zeroes both the stale prefix
                and the intra-tile upper triangle of pt before the full-width
                AV matmul.
                """
                cols = slice(g * 512, (g + 1) * 512)
                for h in range(HPC):
                    db, po = h // 2, (h % 2) * DK
                    av = av_ps.tile([DK + 1, 512], f32, tag="av", name="av")
                    nt = 4 * g + 4
                    for t in range(nt):
                        v = t - 4 * g
                        c0 = 128 * v if v >= 0 else 0
                        sc = sc_ps.tile([P, 512], f32, tag="sc", name="sc")
                        nc.tensor.matmul(
                            sc[:, c0:],
                            kt_sb[db][po:po + DK, t * P:(t + 1) * P],
                            qt_sb[db][po:po + DK,
                                      g * 512 + c0:(g + 1) * 512],
                            start=True, stop=True)
                        pt = ptp.tile([P, 512], bf16, tag="pt", name="pt")
                        nc.scalar.activation(pt[:, c0:], sc[:, c0:], Exp)
                        if v >= 0:
                            nc.vector.tensor_mul(
                                pt, pt, mask_sb[:, v * 512:(v + 1) * 512])
                        nc.tensor.matmul(
                            av, v_sb[t][:, h * (DK + 1):(h + 1) * (DK + 1)],
                            pt, start=(t == 0), stop=(t == nt - 1))
                    rcp = rcpp.tile([1, 512], f32, tag="rcp", name="rcp")
                    nc.vector.reciprocal(rcp, av[DK:DK + 1, :])
                    rmat = rmatp.tile([DK, 512], f32, tag="rmat", name="rmat")
                    nc.gpsimd.partition_broadcast(rmat, rcp, channels=DK)
                    nc.vector.tensor_mul(ot_sb[db][po:po + DK, cols],
                                         av[0:DK, :], rmat)

            def oproj_group(g):
                for m in range(4 * g, 4 * g + 4):
                    for nb in range(2):
                        ps = mm_ps.tile([P, 512], f32, tag="mm", name="mm")
                        for k in range(4):
                            nc.tensor.matmul(
                                ps, ot_sb[k][:, m * P:(m + 1) * P],
                                wo_sb[k][:, nb * 512:(nb + 1) * 512],
                                start=(k == 0), stop=(k == 3))
                        og = ostg.tile([P, 512], bf16, tag="og", name="og")
                        nc.vector.tensor_copy(og, ps)
                        nc.sync.dma_start(
                            out=out[m * P:(m + 1) * P,
                                    nb * 512:(nb + 1) * 512],
                            in_=og)

            # software-staged emission: keep PE fed with proj work while the
            # ACT-heavy attention of earlier groups drains
            proj_group(0)
            proj_group(1)
            for g in range(4):
                attn_group(g)
                if g + 2 < 4:
                    proj_group(g + 2)
                oproj_group(g)

        for _ in range(reps):
            emit_body()

    nc.compile()
    return nc


def _get_nc(reps=1):
    key = ("nc", reps)
    if key not in _CACHE:
        _CACHE[key] = _build(reps)
    return _CACHE[key]


def _prep_core_inputs(q_proj_weight, k_proj_weight, v_proj_weight,
                      o_proj_weight, in_features, token_positions):
    """Host-side sharding: returns the list of 8 per-core input dicts."""
    import ml_dtypes
    bf = ml_dtypes.bfloat16

    x = np.asarray(in_features, np.float32)
    wqf = np.asarray(q_proj_weight, np.float32)
    wkf = np.asarray(k_proj_weight, np.float32)
    wvf = np.asarray(v_proj_weight, np.float32)
    wof = np.asarray(o_proj_weight, np.float32)
    tp = np.asarray(token_positions).astype(np.float64)

    inv = 1.0 / (THETA ** (np.arange(HALF, dtype=np.float64) / HALF))
    fr = tp[:, None] * inv[None, :]                       # [S, 32]
    cosn = np.repeat(np.cos(fr), 2, axis=1).astype(bf)    # [S, 64]
    sg = np.tile(np.array([-1.0, 1.0]), HALF)[None, :]
    sinn = (np.repeat(np.sin(fr), 2, axis=1) * sg).astype(bf)

    kv = np.arange(P)[:, None]
    qc = np.arange(512)[None, :]
    maskt = np.concatenate(
        [(qc >= 128 * v + kv) for v in range(4)], axis=1).astype(bf)

    identity = np.eye(P, dtype=bf)

    in_maps = []
    for c in range(N_CORES):
        b, hg = c // 2, c % 2
        rows = slice(hg * DOUT, (hg + 1) * DOUT)
        blob = np.empty(BLOB_N, dtype=bf)

        def put(name, arr):
            fl = np.ascontiguousarray(arr, dtype=bf).reshape(-1)
            blob[_OFF[name]:_OFF[name] + fl.size] = fl

        put("xT", x[b].T)
        put("wq", (wqf[rows] * SCALE).T)
        put("wk", wkf[rows].T)
        put("wv", wvf[rows].T)
        put("cos", cosn)
        put("sin", sinn)
        put("mask", maskt)
        put("ident", identity)
        put("wo", wof[:, rows].T)
        in_maps.append({"blob": blob})
    return in_maps


def kernel(q_proj_weight, k_proj_weight, v_proj_weight, o_proj_weight,
           in_features, token_positions):
    from concourse.bass_utils import run_bass_kernel_spmd

    nc = _get_nc()
    in_maps = _prep_core_inputs(q_proj_weight, k_proj_weight, v_proj_weight,
                                o_proj_weight, in_features, token_positions)
    trace = bool(int(os.environ.get("KBENCH_TRACE", "0")))
    res = run_bass_kernel_spmd(nc, in_maps, list(range(N_CORES)), trace=trace)
    _CACHE["last_results"] = res
    if res.exec_time_ns is not None:
        _CACHE["exec_time_ns"] = res.exec_time_ns

    outp = np.empty((B, S, D_MODEL), np.float32)
    for b in range(B):
        outp[b] = (res.results[2 * b]["out"].astype(np.float32)
                   + res.results[2 * b + 1]["out"].astype(np.float32))
    return outp
